# revision 43
# baseline (speedup 1.0000x reference)
"""Trainium2 Bass kernel for CRF negative-log-likelihood loss.

nn_CRF (B=512, L=1024, T=48), data-parallel over 8 NeuronCores (Bc=64
rows per core); host sums the 8 scalar partials.

Design (v2, segmented scan):
  Forward (partition function): the linear-domain scan
  A_t = (E^T A_{t-1}) * F_t (E = exp(trans - log T), F = exp(feat - MU))
  is split into NSEG=24 time segments of SEGLEN=43 steps with W=12
  warmup steps each (Hilbert-metric contraction of E makes the
  direction forget its init in ~8 steps, and diagonal F scalings are
  Hilbert isometries, so a warm-started segment converges to the true
  forward direction up to a per-column scale).  No renormalisation is
  needed inside a segment: fp32/bf16 exponent range absorbs the drift,
  and per-segment log-gains telescope through boundary column sums.
  Segments are packed 2-per-partition-group x 6-per-column-group into
  C=2 independent chains of (112, 384) matmul+multiply steps, with an
  exact side chain (48, 64) covering t in [0, 56) to anchor the
  telescoped magnitude.  The stationary matrix carries extra columns
  that compute end-capture rows and column sums for free; those rows
  ride through the F-multiply (F rows 48-63 are 1.0 via natfb padding)
  and are staged to DRAM, reloaded in (slot, seg*batch) layout, and
  selected by per-row length indicators.
  F tiles are produced by XBAR DMA transposes (128-source-column tiles)
  of a pre-exponentiated, 64-element-padded bf16 copy of feats (natfb),
  so the PE does no transposes and the ACT does no PSUM evacuation.
  Gold (numerator): one-hot tiles (bf16 tensor_scalar is_equal) over
  127-step chunks; bigram counts via offset-partition matmuls
  C += ohu[0:127]^T @ ohu[1:128] (mask baked into tags, so the pair
  weight mask_t*mask_{t+1} = mask_{t+1} is automatic); feature gathers
  via fused scalar_tensor_tensor ops split across DVE and Pool.
"""

import math

import numpy as np

import concourse.bacc as bacc
import concourse.mybir as mybir
import concourse.tile as tile
from concourse.bass import AP
from concourse.bass_utils import run_bass_kernel_spmd

F32 = mybir.dt.float32
BF16 = mybir.dt.bfloat16
I32 = mybir.dt.int32
AF = mybir.ActivationFunctionType
OP = mybir.AluOpType

B_FULL = 512
N_CORES = 8
BC = B_FULL // N_CORES  # 64
L_FULL = 1024
T = 48

MU = 0.51
ALPHA = math.log(T)
SEGLEN = 43
NSEG = 24          # segments s = 1..24, seg s main range [43s, 43s+43)
W = 12             # warmup steps
SLOTS = 56         # tau = 0..55; mm steps tau = 1..56
NTB = 576          # natfb window width in t-slots
W0 = 0             # chain-0 window start (t)
W1 = 508           # chain-1 window start (t)
NCH = 6            # column groups (m) per chain
WCH = NCH * BC     # 384 = chain column width
GOLD_CK = 8        # 128-step gold chunks


def build_program(dbg=False):
    L = L_FULL
    Bc = BC
    nc = bacc.Bacc("TRN2", target_bir_lowering=False, debug=False)

    feats_d = nc.dram_tensor("feats", (Bc, L, T), F32, kind="ExternalInput")
    trans_d = nc.dram_tensor("trans", (T, T), F32, kind="ExternalInput")
    start_d = nc.dram_tensor("start", (T,), F32, kind="ExternalInput")
    end_d = nc.dram_tensor("end", (T,), F32, kind="ExternalInput")
    tags_d = nc.dram_tensor("tags", (Bc, L), I32, kind="ExternalInput")
    mask_d = nc.dram_tensor("mask", (Bc, L), I32, kind="ExternalInput")
    out_d = nc.dram_tensor("out", (1, 1), F32, kind="ExternalOutput")
    dbg_d = (nc.dram_tensor("dbg", (8, Bc), F32, kind="ExternalOutput")
             if dbg else None)

    feats_flat = feats_d.ap().rearrange("b l t -> b (l t)")

    with tile.TileContext(nc) as tc:
        with (
            tc.tile_pool(name="const", bufs=1) as cp,
            tc.tile_pool(name="cps", bufs=1, space="PSUM") as cpp,
            tc.tile_pool(name="dramp", bufs=1, space="DRAM") as dp,
        ):
            # ---------------- constants ----------------
            iota48i = cp.tile((128, T), I32)
            nc.gpsimd.iota(iota48i[:, :], [[1, T]], channel_multiplier=0)
            iota48f = cp.tile((128, T), F32)
            nc.vector.tensor_copy(iota48f[:, :], iota48i[:, :])
            iota48b = cp.tile((128, T), BF16)
            nc.vector.tensor_copy(iota48b[:, :], iota48i[:, :])

            iota64i = cp.tile((64, 64), I32)
            nc.gpsimd.iota(iota64i[:, :], [[1, 64]], channel_multiplier=0)
            iotaPi = cp.tile((64, 1), I32)
            nc.gpsimd.iota(iotaPi[:, :], [[1, 1]], channel_multiplier=1)
            iota64f = cp.tile((64, 64), F32)
            nc.vector.tensor_copy(iota64f[:, :], iota64i[:, :])
            iotaPf = cp.tile((64, 1), F32)
            nc.vector.tensor_copy(iotaPf[:, :], iotaPi[:, :])
            identMf = cp.tile((64, 64), F32)
            nc.vector.tensor_scalar(
                identMf[:, :], iota64f[:, :], iotaPf[:, :], None, OP.is_equal)
            identMb = cp.tile((64, 64), BF16)
            nc.vector.tensor_copy(identMb[:, :], identMf[:, :])

            iotaLf = cp.tile((Bc, L), F32)

            ones128f = cp.tile((128, 1), F32)
            nc.vector.memset(ones128f[:, :], 1.0)
            ones128b = cp.tile((128, 1), BF16)
            nc.vector.memset(ones128b[:, :], 1.0)

            bias_mu = cp.tile((128, 1), F32)
            nc.vector.memset(bias_mu[:, :], -MU)
            bias_a = cp.tile((T, 1), F32)
            nc.vector.memset(bias_a[:, :], -ALPHA)

            # ---------------- params ----------------
            trans_sb = cp.tile((T, T), F32)
            nc.sync.dma_start(trans_sb[:, :], trans_d.ap())
            e_f32 = cp.tile((T, T), F32)
            nc.scalar.activation(e_f32[:, :], trans_sb[:, :], AF.Exp,
                                 bias=bias_a[:, :])
            e_b = cp.tile((T, T), BF16)
            nc.vector.tensor_copy(e_b[:, :], e_f32[:, :])

            end_sb = cp.tile((T, 1), F32)
            nc.sync.dma_start(end_sb[:, :], end_d.ap().unsqueeze(1))
            expend_f = cp.tile((T, 1), F32)
            nc.scalar.activation(expend_f[:, :], end_sb[:, :], AF.Exp)
            expend_b = cp.tile((T, 1), BF16)
            nc.vector.tensor_copy(expend_b[:, :], expend_f[:, :])

            start_sb = cp.tile((T, 1), F32)
            nc.sync.dma_start(start_sb[:, :], start_d.ap().unsqueeze(1))
            expstart = cp.tile((T, 1), F32)
            nc.scalar.activation(expstart[:, :], start_sb[:, :], AF.Exp)

            startbc = cp.tile((Bc, T), F32)
            nc.sync.dma_start(
                startbc[:, :], start_d.ap().unsqueeze(0).partition_broadcast(Bc))
            endbc = cp.tile((Bc, T), F32)
            nc.sync.dma_start(
                endbc[:, :], end_d.ap().unsqueeze(0).partition_broadcast(Bc))

            # main stationary (112, 112):
            #  rows 0-47 (block A), rows 64-111 (block B), rows 48-63 zero
            #  cols 0-47 = E(A), 64-111 = E(B), 48 = capA, 49 = capB,
            #  50 = sumA, 51 = sumB, 52-63 zero
            s_main = cp.tile((112, 112), BF16)
            nc.vector.memset(s_main[:, :], 0.0)
            nc.vector.tensor_copy(s_main[0:T, 0:T], e_b[:, :])
            nc.sync.dma_start(s_main[64:112, 64:112], e_b[:, :])
            nc.vector.tensor_copy(s_main[0:T, 48:49], expend_b[:, :])
            nc.sync.dma_start(s_main[64:112, 49:50], expend_b[:, :])
            nc.vector.memset(s_main[0:T, 50:51], 1.0)
            nc.vector.memset(s_main[64:112, 51:52], 1.0)

            # side stationary (48, 50): cols 0-47 E, 48 = cap, 49 = sum
            s_side = cp.tile((T, 50), BF16)
            nc.vector.tensor_copy(s_side[:, 0:T], e_b[:, :])
            nc.vector.tensor_copy(s_side[:, 48:49], expend_b[:, :])
            nc.vector.memset(s_side[:, 49:50], 1.0)

            # ---------------- tags / mask prep ----------------
            prep_scope = tc.tile_pool(name="prepsb", bufs=1)
            prp = prep_scope.__enter__()
            iotaLi = prp.tile((Bc, L), I32)
            nc.gpsimd.iota(iotaLi[:, :], [[1, L]], channel_multiplier=0)
            nc.vector.tensor_copy(iotaLf[:, :], iotaLi[:, :])
            tags_i = prp.tile((Bc, L), I32)
            nc.sync.dma_start(tags_i[:, :], tags_d.ap())
            tagsf = cp.tile((Bc, L), F32)
            nc.vector.tensor_copy(tagsf[:, :], tags_i[:, :])
            mask_i = prp.tile((Bc, L), I32)
            nc.sync.dma_start(mask_i[:, :], mask_d.ap())
            maskf = prp.tile((Bc, L), F32)
            nc.vector.tensor_copy(maskf[:, :], mask_i[:, :])
            tagsmb = prp.tile((Bc, L), BF16)
            moff = prp.tile((Bc, L), F32)
            nc.vector.tensor_scalar(moff[:, :], maskf[:, :], -100.0, 100.0,
                                    OP.mult, OP.add)
            tagsm_f = prp.tile((Bc, L), F32)
            nc.vector.tensor_tensor(tagsm_f[:, :], tagsf[:, :], moff[:, :],
                                    OP.add)
            nc.vector.tensor_copy(tagsmb[:, :], tagsm_f[:, :])

            tagsmSb = prp.tile((Bc, L), BF16)
            nc.vector.memset(tagsmSb[:, :], 100.0)
            nc.vector.tensor_copy(tagsmSb[:, 0:L - 1], tagsm_f[:, 1:L])

            lenb = cp.tile((Bc, 1), F32)
            nc.vector.tensor_reduce(lenb[:, :], maskf[:, :],
                                    mybir.AxisListType.X, OP.add)

            # transposed masked tags: 9 tiles (128, 64), 127-stride chunks
            tagsTm = []
            tagsTmS = []
            with tc.tile_pool(name="prepps", bufs=2, space="PSUM") as ppp:
                for k in range(GOLD_CK):
                    ps = ppp.tile((128, Bc), BF16, name=f"tps_{k}", tag="tps",
                                  bufs=2)
                    nc.tensor.transpose(ps[:, :],
                                        tagsmb[:, 128 * k:128 * (k + 1)],
                                        identMb[:, :])
                    tt = cp.tile((128, Bc), BF16, name=f"tagsTm_{k}")
                    nc.vector.tensor_copy(tt[:, :], ps[:, :])
                    tagsTm.append(tt)
                    ps2 = ppp.tile((128, Bc), BF16, name=f"tps2_{k}",
                                   tag="tps", bufs=2)
                    nc.tensor.transpose(ps2[:, :],
                                        tagsmSb[:, 128 * k:128 * (k + 1)],
                                        identMb[:, :])
                    tt2 = cp.tile((128, Bc), BF16, name=f"tagsTmS_{k}")
                    nc.vector.tensor_copy(tt2[:, :], ps2[:, :])
                    tagsTmS.append(tt2)
                # len row (1, 64) via transpose
                lps = ppp.tile((1, Bc), F32, name="lps", tag="lps", bufs=1)
                nc.tensor.transpose(lps[:, :], lenb[:, :], identMf[:, :])
                lenrow = cp.tile((1, Bc), F32)
                nc.vector.tensor_copy(lenrow[:, :], lps[:, :])
            prep_scope.__exit__(None, None, None)

            # (natfb/natfbS/A tiles are allocated inside the scan scope
            # below so their SBUF frees before the end phase)
            # ---------------- natfb: padded exp'd bf16 feats ----------------
            # (128, 56*12*64): row c*64+b holds chain c; column layout
            # (tau*12 + strip)*64 + jj with strip = m*2 + tp, so each
            # XBAR transpose slab input is CONTIGUOUS and 128-element
            # source groups give partitions tp*64 + jj.  jj 48-63 = 1.0
            # (become the F=1 ride-through rows after transpose).
            HA, HB = 32, 24  # natfb tau-halves: A = tau [0,32), B = [32,56)
            bigp_scope = tc.tile_pool(name="bigp", bufs=1)
            bigp = bigp_scope.__enter__()
            natfbA = bigp.tile((128, HA * 12 * 64), BF16)
            natfbB = bigp.tile((128, HB * 12 * 64), BF16)
            for nt, nh in ((natfbA, HA), (natfbB, HB)):
                nc.gpsimd.memset(
                    nt[:, :].rearrange("p (ts jj) -> p ts jj", ts=nh * 12,
                                       jj=64)[:, :, T:64], 1.0)
            # invalid tails of the two clipped strips (chain 1, s=23, 24)
            # strip (tp=1, m=4) -> strip idx 9: slots tau >= 47 invalid
            # strip (tp=1, m=5) -> strip idx 11: slots tau >= 4 invalid
            nc.gpsimd.memset(
                natfbB[64:128, :].rearrange(
                    "p (t s jj) -> p t s jj", t=HB, s=12,
                    jj=64)[:, 47 - HA:HB, 9, 0:T], 1.0)
            nc.gpsimd.memset(
                natfbA[64:128, :].rearrange(
                    "p (t s jj) -> p t s jj", t=HA, s=12,
                    jj=64)[:, 4:HA, 11, 0:T], 1.0)
            nc.gpsimd.memset(
                natfbB[64:128, :].rearrange(
                    "p (t s jj) -> p t s jj", t=HB, s=12,
                    jj=64)[:, :, 11, 0:T], 1.0)
            # side-chain feats: plain t-slot layout, t in [0, 56)
            natfbS = bigp.tile((Bc, SLOTS * 64), BF16)
            nc.gpsimd.memset(
                natfbS[:, :].rearrange("p (t jj) -> p t jj", t=SLOTS,
                                       jj=64)[:, :, T:64], 1.0)

            # stage DRAM: per chain (4, 57*384) bf16
            stage_dr = [dp.tile((4, 57 * WCH), BF16, name=f"stage_{c}")
                        for c in range(2)]

            with (
                tc.tile_pool(name="chkp", bufs=3) as chp,
                tc.tile_pool(name="fslab", bufs=2) as fsp,
                tc.tile_pool(name="fside", bufs=2) as fsdp,
                tc.tile_pool(name="scanps", bufs=3, space="PSUM") as sps,
                tc.tile_pool(name="scanps2", bufs=3, space="PSUM") as sps2,
                tc.tile_pool(name="sideps", bufs=1, space="PSUM") as sdps,
                tc.tile_pool(name="ohp", bufs=3) as ohp,
                tc.tile_pool(name="bounce", bufs=1) as bpp,
                tc.tile_pool(name="fgp", bufs=2) as fgp,
                tc.tile_pool(name="scrp", bufs=2) as scrp,
            ):
                # ---- feats strip DMAs + exp into natfb halves ----
                natfb4 = {}
                natfb4c1 = {}
                for nt, nh, hh in ((natfbA, HA, 0), (natfbB, HB, 1)):
                    natfb4[hh] = nt[:, :].rearrange(
                        "p (t s jj) -> p t s jj", t=nh, s=12, jj=64)
                    natfb4c1[hh] = nt[64:128, :].rearrange(
                        "p (t s jj) -> p t s jj", t=nh, s=12, jj=64)
                HOFF = {0: 0, 1: HA}
                HLEN = {0: HA, 1: HB}

                def emit_strip_half(tp, m, h):
                    # strip = m*2 + tp; seg s_c = 12c + 6tp + m + 1
                    strip = m * 2 + tp
                    s0 = 6 * tp + m + 1
                    s1 = s0 + 12
                    nvalid1 = min(SLOTS, max(0, L - (SEGLEN * s1 - 12)))
                    hl = HLEN[h]
                    t0 = SEGLEN * s0 - 12 + HOFF[h]
                    if nvalid1 == SLOTS:
                        ch = chp.tile((128, HA * T), F32, name="natf")
                        in_ap = AP(feats_flat.tensor, t0 * T,
                                   [[516 * T, 2], [L * T, Bc],
                                    [1, hl * T]])
                        nc.sync.dma_start(
                            ch[:, 0:hl * T].rearrange(
                                "p (a b) -> p a b", a=1, b=hl * T), in_ap)
                        nc.scalar.activation(
                            natfb4[h][:, :, strip, 0:T],
                            ch[:, 0:hl * T], AF.Exp, bias=bias_mu[:, :])
                    else:
                        ch = chp.tile((128, HA * T), F32, name="natf")
                        in_ap = AP(feats_flat.tensor, t0 * T,
                                   [[L * T, Bc], [1, hl * T]])
                        nc.sync.dma_start(ch[0:Bc, 0:hl * T], in_ap)
                        nc.scalar.activation(
                            natfb4[h][0:Bc, :, strip, 0:T],
                            ch[0:Bc, 0:hl * T], AF.Exp,
                            bias=bias_mu[0:Bc, :])
                        nv = min(max(nvalid1 - HOFF[h], 0), hl)
                        if nv > 0:
                            ch2 = chp.tile((128, HA * T), F32,
                                           name="natf")
                            in2 = AP(feats_flat.tensor,
                                     (SEGLEN * s1 - 12 + HOFF[h]) * T,
                                     [[L * T, Bc], [1, nv * T]])
                            nc.sync.dma_start(ch2[0:Bc, 0:nv * T], in2)
                            nc.scalar.activation(
                                natfb4c1[h][:, 0:nv, strip, 0:T],
                                ch2[0:Bc, 0:nv * T], AF.Exp,
                                bias=bias_mu[0:Bc, :])

                # side strip first (unblocks the side chain)
                for h in range(2):
                    HSs = SLOTS // 2
                    chS = chp.tile((128, HA * T), F32, name="natf")
                    nc.sync.dma_start(
                        chS[0:Bc, 0:HSs * T],
                        AP(feats_flat.tensor, h * HSs * T,
                           [[L * T, Bc], [1, HSs * T]]))
                    nc.scalar.activation(
                        natfbS[:, :].rearrange(
                            "p (t jj) -> p t jj", t=SLOTS,
                            jj=64)[:, h * HSs:(h + 1) * HSs, 0:T],
                        chS[0:Bc, 0:HSs * T], AF.Exp,
                        bias=bias_mu[0:Bc, :])
                for h in range(2):
                    for m in range(NCH):
                        for tp in range(2):
                            emit_strip_half(tp, m, h)

                # ---- side chain (exact, t in [0, 56]) ----
                # F side slabs: q covers tau in [8q, 8q+8)
                side_slabs = {}

                natfbS_t = natfbS[:, :].tensor

                def emit_side_slab(q):
                    sl = fsdp.tile((128, 4 * 64), BF16, name="fside")
                    in_ap = AP(natfbS_t, 8 * q * 64,
                               [[SLOTS * 64, Bc], [1, 512]])
                    nc.scalar.dma_start_transpose(
                        sl[:, :].rearrange("p (e b) -> p e b", e=4, b=64),
                        in_ap)
                    side_slabs[q] = sl

                def side_f(tau):
                    sl = side_slabs[tau // 8]
                    p0 = (tau % 2) * 64
                    c0 = ((tau // 2) % 4) * 64
                    return sl[p0:p0 + 50, c0:c0 + 64]

                emit_side_slab(0)
                emit_side_slab(1)

                side_pool = tc.tile_pool(name="sidea", bufs=3)
                sap = side_pool.__enter__()
                a_side = sap.tile((50, Bc), BF16, name="a_side")
                # A_side(0) = exp(start) * F_0  (rows 48-49 will be junk)
                nc.vector.memset(a_side[32:50, :], 1.0)
                nc.vector.tensor_scalar(a_side[0:T, :],
                                        side_slabs[0][0:T, 0:64],
                                        expstart[:, :], None, OP.mult)
                lnsideS43 = cp.tile((1, Bc), F32)
                sums_side44 = cp.tile((1, Bc), BF16)

                for tau in range(1, 45):
                    if tau % 8 == 1 and tau // 8 + 2 <= 5:
                        emit_side_slab(tau // 8 + 2)
                    ps = sdps.tile((50, Bc), F32, name="side_ps")
                    nc.tensor.matmul(ps[:, :], s_side[:, :], a_side[0:T, :],
                                     start=True, stop=True,
                                     skip_group_check=True)
                    a_new = sap.tile((50, Bc), BF16, name="a_side")
                    nc.vector.tensor_tensor(a_new[:, :], ps[:, :],
                                            side_f(tau), OP.mult)
                    if tau == 44:
                        nc.sync.dma_start(sums_side44[:, :],
                                          a_new[49:50, :])
                    a_side = a_new
                nc.scalar.activation(lnsideS43[:, :], sums_side44[:, :],
                                     AF.Ln)
                side_pool.__exit__(None, None, None)

                # ---- main F slabs ----
                main_slabs = {}
                natfbA_t = natfbA[:, :].tensor
                natfbB_t = natfbB[:, :].tensor

                def emit_main_slab(c, q):
                    sl = fsp.tile((128, 8 * WCH), BF16, name="fslab")
                    nt, nh, qoff = ((natfbA_t, HA, 0) if q < 4
                                    else (natfbB_t, HB, 4))
                    if c == 0:
                        in_ap = AP(nt, (q - qoff) * 8 * 768,
                                   [[nh * 12 * 64, Bc], [1, 8 * 768]])
                    else:
                        # XBAR input must start at partition 0: bounce
                        # the chain-1 span down via an SBUF DMA first
                        bt = bpp.tile((Bc, 8 * 768), BF16, name="bounce")
                        nc.sync.dma_start(
                            bt[:, :],
                            AP(nt, 64 * (nh * 12 * 64) + (q - qoff)
                               * 8 * 768,
                               [[nh * 12 * 64, Bc], [1, 8 * 768]]))
                        in_ap = bt[:, :]
                    teng = nc.scalar if (c + q) % 2 == 0 else nc.sync
                    teng.dma_start_transpose(
                        sl[:, :].rearrange("p (e b) -> p e b", e=8 * NCH,
                                           b=64),
                        in_ap)
                    main_slabs[(c, q)] = sl

                for c in range(2):
                    emit_main_slab(c, 0)
                    emit_main_slab(c, 1)

                # ---- A ping-pong tiles ----
                app = [[bigp.tile((112, 8 * WCH), BF16, name=f"A_{c}_{i}")
                        for i in range(2)] for c in range(2)]
                for c in range(2):
                    t0 = app[c][0]
                    nc.vector.memset(t0[0:64, 0:WCH], 0.0)
                    nc.vector.memset(t0[0:52, 0:WCH], 1.0)
                    nc.vector.memset(t0[64:112, 0:WCH], 1.0)

                # ---- gold work generator (interleaved) ----
                c_ps = cpp.tile((T, T), F32, name="c_ps")
                feat_acc = cp.tile((128, 64), F32)
                nc.vector.memset(feat_acc[:, :], 0.0)
                # b-major iota: val[p, b*48+j] = j
                iota384i = cp.tile((128, 384), I32)
                nc.gpsimd.iota(iota384i[:, :], [[0, 8], [1, T]],
                               channel_multiplier=0)
                iota384b = cp.tile((128, 384), BF16)
                nc.vector.tensor_copy(iota384b[:, :], iota384i[:, :])

                gold_units = [(o, k) for o in range(8)
                              for k in range(GOLD_CK)]
                n_units = len(gold_units)
                gold_pos = [0]
                first_c = [True]

                def emit_gold(n):
                    for _ in range(n):
                        u = gold_pos[0]
                        if u >= n_units:
                            return
                        o, k = gold_units[u]
                        fg8 = fgp.tile((128, 384), F32, name="fg8")
                        in_ap = AP(feats_flat.tensor,
                                   8 * o * L * T + 128 * k * T,
                                   [[T, 128], [L * T, 8], [1, T]])
                        nc.gpsimd.dma_start(fg8[:, :], in_ap)
                        ohu8 = ohp.tile((128, 384), BF16, name="ohu8")
                        tu = tagsTm[k][:, :].tensor
                        nc.vector.tensor_tensor(
                            ohu8[:, :], iota384b[:, :],
                            AP(tu, 8 * o, [[Bc, 128], [1, 8], [0, T]]),
                            OP.is_equal)
                        ohs8 = ohp.tile((128, 384), BF16, name="ohs8")
                        ts_ = tagsTmS[k][:, :].tensor
                        nc.vector.tensor_tensor(
                            ohs8[:, :], iota384b[:, :],
                            AP(ts_, 8 * o, [[Bc, 128], [1, 8], [0, T]]),
                            OP.is_equal)
                        for b in range(8):
                            nc.tensor.matmul(
                                c_ps[:, :],
                                ohu8[:, b * T:(b + 1) * T],
                                ohs8[:, b * T:(b + 1) * T],
                                start=first_c[0], stop=False,
                                skip_group_check=True)
                            first_c[0] = False
                        scr = scrp.tile((128, 384), F32, name="scr",
                                        tag="scr")
                        nc.vector.scalar_tensor_tensor(
                            scr[:, :], ohu8[:, :], 1.0, fg8[:, :],
                            OP.mult, OP.mult,
                            accum_out=feat_acc[:, u:u + 1])
                        gold_pos[0] += 1

                # ---- main scan ----
                def a_slice(c, tau):
                    return app[c][(tau // 8) % 2][:, (tau % 8) * WCH:
                                                  (tau % 8 + 1) * WCH]

                def f_slice(c, tau):
                    # step 56 only needs the F=1 ride-through rows; reuse
                    # slot 55 (A rows 0-111 of state 56 are never used)
                    tau = min(tau, SLOTS - 1)
                    sl = main_slabs[(c, tau // 8)]
                    return sl[0:112, (tau % 8) * WCH:(tau % 8 + 1) * WCH]

                emit_gold(6)
                for tau in range(1, SLOTS + 1):
                    if tau % 8 == 1:
                        q = tau // 8 + 2
                        if q <= 6:
                            for c in range(2):
                                emit_main_slab(c, q)
                    for c in range(2):
                        pool = sps if c == 0 else sps2
                        ps = pool.tile((112, WCH), F32, name=f"mm_{c}")
                        nc.tensor.matmul(ps[:, :], s_main[:, :],
                                         a_slice(c, tau - 1)[0:112, :],
                                         start=True, stop=True,
                                         skip_group_check=True)
                        nc.vector.tensor_tensor(a_slice(c, tau), ps[:, :],
                                                f_slice(c, tau), OP.mult)
                    if tau % 8 == 7 or tau == SLOTS:
                        pass
                    if tau % 8 == 0:
                        q = tau // 8 - 1
                        for c in range(2):
                            nc.sync.dma_start(
                                stage_dr[c][0:4,
                                            q * 8 * WCH:(q + 1) * 8 * WCH],
                                app[c][q % 2][48:52, :])
                    emit_gold(2)
                # final partial block: slot 56 = A(56)
                for c in range(2):
                    nc.sync.dma_start(
                        stage_dr[c][0:4, 56 * WCH:57 * WCH],
                        app[c][1][48:52, 0:WCH])
                emit_gold(n_units)
                zrow = cp.tile((1, T), BF16)
                nc.vector.memset(zrow[:, :], 0.0)
                nc.tensor.matmul(c_ps[:, :], zrow[:, :], zrow[:, :],
                                 start=False, stop=True,
                                 skip_group_check=True)

                # ---- gold misc terms (start, end-transitions) ----
                misc_acc = cp.tile((Bc, 2), F32)
                scrb = scrp.tile((Bc, T), F32, name="scrb", tag="scrb")
                nc.vector.scalar_tensor_tensor(
                    scrb[:, :], iota48f[0:Bc, :], tagsf[:, 0:1],
                    startbc[:, :], OP.is_equal, OP.mult,
                    accum_out=misc_acc[:, 0:1])
                lm1 = cp.tile((Bc, 1), F32)
                nc.vector.tensor_scalar(lm1[:, :], lenb[:, :], 1.0, None,
                                        OP.subtract)
                scrLt = chp.tile((128, HA * T), F32, name="natf")
                scrL = scrLt[0:Bc, 0:L]
                lt = cp.tile((Bc, 1), F32)
                nc.vector.scalar_tensor_tensor(
                    scrL, iotaLf[:, :], lm1[:, :], tagsf[:, :],
                    OP.is_equal, OP.mult, accum_out=lt[:, :])
                scrb3 = scrp.tile((Bc, T), F32, name="scrb3", tag="scrb")
                nc.vector.scalar_tensor_tensor(
                    scrb3[:, :], iota48f[0:Bc, :], lt[:, :], endbc[:, :],
                    OP.is_equal, OP.mult, accum_out=misc_acc[:, 1:2])

            bigp_scope.__exit__(None, None, None)

            # =============== end phase ===============
            with (
                tc.tile_pool(name="endp", bufs=1) as ep,
                tc.tile_pool(name="endps", bufs=1, space="PSUM") as epp,
                tc.tile_pool(name="endps2", bufs=2, space="PSUM") as epp2,
            ):
                # gold assembly
                gold_ps = epp.tile((1, 1), F32, name="gold_ps")
                scrT = ep.tile((T, T), F32, name="scrT")
                cacc = ep.tile((T, 1), F32, name="cacc")
                nc.vector.tensor_tensor(scrT[:, :], c_ps[:, :],
                                        trans_sb[:, :], OP.mult)
                nc.vector.tensor_reduce(cacc[:, :], scrT[:, :],
                                        mybir.AxisListType.X, OP.add)
                nc.tensor.matmul(gold_ps[:, :], ones128f[0:T, :], cacc[:, :],
                                 start=True, stop=False,
                                 skip_group_check=True)
                fred = ep.tile((128, 1), F32, name="fred")
                nc.vector.tensor_reduce(fred[:, :], feat_acc[:, :],
                                        mybir.AxisListType.X, OP.add)
                nc.tensor.matmul(gold_ps[:, :], ones128f[:, :],
                                 fred[:, :], start=False, stop=False,
                                 skip_group_check=True)
                mred = ep.tile((Bc, 1), F32, name="mred")
                nc.vector.tensor_reduce(mred[:, :], misc_acc[:, :],
                                        mybir.AxisListType.X, OP.add)
                nc.tensor.matmul(gold_ps[:, :], ones128f[0:Bc, :],
                                 mred[:, :], start=False, stop=True,
                                 skip_group_check=True)

                # reload caps/sums: (57, 768) per chain
                capsre, sumsre, capsLn, sumsLn = [], [], [], []
                for c in range(2):
                    cr = ep.tile((57, 2 * WCH), BF16, name=f"capsre_{c}")
                    sr = ep.tile((57, 2 * WCH), BF16, name=f"sumsre_{c}")
                    st_t = stage_dr[c][:, :].tensor
                    nc.sync.dma_start(
                        cr[:, :], AP(st_t, 0,
                                     [[WCH, 57], [57 * WCH, 2],
                                      [64, NCH], [1, 64]]))
                    nc.sync.dma_start(
                        sr[:, :], AP(st_t, 2 * 57 * WCH,
                                     [[WCH, 57], [57 * WCH, 2],
                                      [64, NCH], [1, 64]]))
                    cl = ep.tile((57, 2 * WCH), F32, name=f"capsLn_{c}")
                    sl_ = ep.tile((57, 2 * WCH), F32, name=f"sumsLn_{c}")
                    nc.scalar.activation(cl[:, :], cr[:, :], AF.Ln)
                    nc.scalar.activation(sl_[:, :], sr[:, :], AF.Ln)
                    capsre.append(cr)
                    sumsre.append(sr)
                    capsLn.append(cl)
                    sumsLn.append(sl_)

                # lenrep (1, 1536) f32
                lenrep = ep.tile((1, 24 * 64), F32, name="lenrep")
                nc.vector.tensor_copy(lenrep[:, 0:64], lenrow[:, :])
                for w_ in (64, 128, 256, 512):
                    nc.vector.tensor_copy(lenrep[:, w_:2 * w_],
                                          lenrep[:, 0:w_])
                nc.vector.tensor_copy(lenrep[:, 1024:1536],
                                      lenrep[:, 0:512])

                # per-chain sigma indicator + capture select
                ones_row = ep.tile((1, 64), F32, name="ones_row")
                nc.vector.memset(ones_row[:, :], 1.0)
                comb = ep.tile((1, 24 * 64), F32, name="comb")
                for c in range(2):
                    io = ep.tile((57, 2 * WCH), I32, name=f"indio_{c}")
                    nc.gpsimd.iota(io[:, :], [[258, 2], [43, NCH], [0, 64]],
                                   channel_multiplier=1)
                    iof = ep.tile((57, 2 * WCH), F32,
                                  name=f"indiof_{c}")
                    nc.vector.tensor_copy(iof[:, :], io[:, :])
                    nc.vector.memset(iof[0:13, :], -9999.0)
                    if c == 0:
                        negrow = ep.tile((1, 2 * WCH), F32, name="negrow")
                        nc.vector.memset(negrow[:, :], -9999.0)
                    nc.sync.dma_start(iof[56:57, :], negrow[:, :])
                    lr_c = ep.tile((1, 2 * WCH), F32, name=f"lrc_{c}")
                    nc.vector.tensor_scalar(
                        lr_c[:, :], lenrep[:, 0:2 * WCH],
                        float(31 + 516 * c), None, OP.subtract)
                    for h in range(2):
                        hs = slice(h * WCH, (h + 1) * WCH)
                        lps_c = epp2.tile((57, WCH), F32,
                                          name=f"lps_{c}_{h}", tag="lps",
                                          bufs=2)
                        nc.tensor.matmul(lps_c[:, :], ones_row[0:1, 0:57],
                                         lr_c[:, hs], start=True, stop=True,
                                         skip_group_check=True)
                        ind = ep.tile((57, WCH), F32,
                                      name=f"ind_{c}_{h}")
                        nc.vector.tensor_tensor(ind[:, :], iof[:, hs],
                                                lps_c[:, :], OP.is_equal)
                        pr = ep.tile((57, WCH), F32,
                                      name=f"pr_{c}_{h}")
                        nc.vector.tensor_tensor(pr[:, :], capsLn[c][:, hs],
                                                ind[:, :], OP.mult)
                        fsel = epp2.tile((1, WCH), F32,
                                         name=f"fsel_{c}_{h}", tag="fsel",
                                         bufs=2)
                        nc.tensor.matmul(fsel[:, :], ones128f[0:57, :],
                                         pr[:, :], start=True, stop=True,
                                         skip_group_check=True)
                        nc.vector.tensor_copy(
                            comb[:, c * 768 + h * WCH:
                                 c * 768 + (h + 1) * WCH], fsel[:, :])
                Gall = ep.tile((1, 24 * 64), F32, name="Gall")
                s13 = ep.tile((1, 24 * 64), F32, name="s13")
                s56 = ep.tile((1, 24 * 64), F32, name="s56")
                for c in range(2):
                    nc.sync.dma_start(s13[:, c * 768:(c + 1) * 768],
                                      sumsLn[c][13:14, :])
                    nc.sync.dma_start(s56[:, c * 768:(c + 1) * 768],
                                      sumsLn[c][56:57, :])
                nc.vector.tensor_tensor(Gall[:, :], s56[:, :], s13[:, :],
                                        OP.subtract)
                # ge[s] = (len-1 >= 43(s+1)) at pos (s, b)
                ioS = ep.tile((1, 24 * 64), I32, name="ioS")
                nc.gpsimd.iota(ioS[:, :], [[43, 24], [0, 64]],
                               channel_multiplier=0)
                ioSf = ep.tile((1, 24 * 64), F32, name="ioSf")
                nc.vector.tensor_copy(ioSf[:, :], ioS[:, :])
                # ge1[g] = (s* >= g+1), ge2[g] = (s* >= g+2) with
                # s* = (len-1)//43; include G of seg s=g+1 iff ge2;
                # one-hot of s* group = ge1 - ge2
                lm44 = ep.tile((1, 24 * 64), F32, name="lm44")
                nc.vector.tensor_scalar(lm44[:, :], lenrep[:, :], 44.0,
                                        None, OP.subtract)
                ge1 = ep.tile((1, 24 * 64), F32, name="ge1")
                nc.vector.tensor_tensor(ge1[:, :], lm44[:, :], ioSf[:, :],
                                        OP.is_ge)
                lm87 = ep.tile((1, 24 * 64), F32, name="lm87")
                nc.vector.tensor_scalar(lm87[:, :], lenrep[:, :], 87.0,
                                        None, OP.subtract)
                ge2 = ep.tile((1, 24 * 64), F32, name="ge2")
                nc.vector.tensor_tensor(ge2[:, :], lm87[:, :], ioSf[:, :],
                                        OP.is_ge)
                ohsel = ep.tile((1, 24 * 64), F32, name="ohsel")
                nc.vector.tensor_tensor(ohsel[:, :], ge1[:, :], ge2[:, :],
                                        OP.subtract)
                tmp = ep.tile((1, 24 * 64), F32, name="tmp")
                nc.vector.tensor_tensor(tmp[:, :], ge2[:, :], Gall[:, :],
                                        OP.mult)
                nc.vector.tensor_tensor(comb[:, :], comb[:, :], tmp[:, :],
                                        OP.add)
                nc.vector.tensor_tensor(tmp[:, :], ohsel[:, :], s13[:, :],
                                        OP.mult)
                nc.vector.tensor_tensor(comb[:, :], comb[:, :], tmp[:, :],
                                        OP.subtract)
                # fold 24 groups -> 1
                for span in (768, 384, 192, 128, 64):
                    if span == 128:
                        nc.vector.tensor_tensor(comb[:, 0:64], comb[:, 0:64],
                                                comb[:, 128:192], OP.add)
                    else:
                        nc.vector.tensor_tensor(comb[:, 0:span],
                                                comb[:, 0:span],
                                                comb[:, span:2 * span],
                                                OP.add)
                # fwd = comb + lnsideS43 + len*(MU+ALPHA) - ALPHA
                fwd = ep.tile((1, Bc), F32, name="fwd")
                nc.vector.tensor_tensor(fwd[:, :], comb[:, 0:64],
                                        lnsideS43[:, :], OP.add)
                shifts = ep.tile((1, Bc), F32, name="shifts")
                nc.vector.tensor_scalar(shifts[:, :], lenrow[:, :],
                                        MU + ALPHA, -ALPHA, OP.mult, OP.add)
                nc.vector.tensor_tensor(fwd[:, :], fwd[:, :], shifts[:, :],
                                        OP.add)
                fwd_tot = ep.tile((1, 1), F32, name="fwd_tot")
                nc.vector.tensor_reduce(fwd_tot[:, :], fwd[:, :],
                                        mybir.AxisListType.X, OP.add)
                loss = ep.tile((1, 1), F32, name="loss")
                nc.vector.tensor_tensor(loss[:, :], fwd_tot[:, :],
                                        gold_ps[:, :], OP.subtract)
                nc.sync.dma_start(out_d.ap(), loss[:, :])
                if dbg:
                    gsb = ep.tile((1, 1), F32, name="gsb")
                    nc.scalar.copy(gsb[:, :], gold_ps[:, :])
                    nc.sync.dma_start(dbg_d.ap()[0:1, :], fwd[:, :])
                    nc.sync.dma_start(dbg_d.ap()[1:2, :], lnsideS43[:, :])
                    nc.sync.dma_start(dbg_d.ap()[2:3, :], lenrow[:, :])
                    nc.sync.dma_start(dbg_d.ap()[3:4, :], comb[:, 0:64])
                    nc.sync.dma_start(dbg_d.ap()[4:5, 0:1], gsb[:, :])
                    nc.sync.dma_start(dbg_d.ap()[5:6, :],
                                      comb[:, 0:64])
                    s44f = ep.tile((1, Bc), F32, name="s44f")
                    nc.vector.tensor_copy(s44f[:, :], sums_side44[:, :])
                    nc.sync.dma_start(dbg_d.ap()[6:7, :], s44f[:, :])
                    nc.sync.dma_start(dbg_d.ap()[7:8, :], lenrep[:, 0:64])

    nc.compile()
    return nc


def shard_inputs(feats, transitions, start_transitions, end_transitions,
                 tags, mask, n_cores=N_CORES):
    feats = np.ascontiguousarray(np.asarray(feats, dtype=np.float32))
    transitions = np.ascontiguousarray(
        np.asarray(transitions, dtype=np.float32))
    start_transitions = np.ascontiguousarray(
        np.asarray(start_transitions, dtype=np.float32))
    end_transitions = np.ascontiguousarray(
        np.asarray(end_transitions, dtype=np.float32))
    tags = np.ascontiguousarray(np.asarray(tags).astype(np.int32))
    mask = np.ascontiguousarray(np.asarray(mask).astype(np.int32))
    Bc = feats.shape[0] // n_cores
    in_maps = []
    for c in range(n_cores):
        s = slice(c * Bc, (c + 1) * Bc)
        in_maps.append({
            "feats": feats[s],
            "trans": transitions,
            "start": start_transitions,
            "end": end_transitions,
            "tags": tags[s],
            "mask": mask[s],
        })
    return in_maps, feats.shape


def kernel(feats, transitions, start_transitions, end_transitions, tags,
           mask, **_ignored):
    in_maps, _ = shard_inputs(
        feats, transitions, start_transitions, end_transitions, tags, mask)
    nc = build_program()
    res = run_bass_kernel_spmd(nc, in_maps, core_ids=list(range(N_CORES)))
    total = sum(float(r["out"][0, 0]) for r in res.results)
    return np.float32(total)


# revision 45
# speedup vs baseline: 1.2188x; 1.2188x over previous
"""Trainium2 Bass kernel for CRF negative-log-likelihood loss.

nn_CRF (B=512, L=1024, T=48), data-parallel over 8 NeuronCores (Bc=64
rows per core); host sums the 8 scalar partials.

Design (v2, segmented scan):
  Forward (partition function): the linear-domain scan
  A_t = (E^T A_{t-1}) * F_t (E = exp(trans - log T), F = exp(feat - MU))
  is split into NSEG=24 time segments of SEGLEN=43 steps with W=12
  warmup steps each (Hilbert-metric contraction of E makes the
  direction forget its init in ~8 steps, and diagonal F scalings are
  Hilbert isometries, so a warm-started segment converges to the true
  forward direction up to a per-column scale).  No renormalisation is
  needed inside a segment: fp32/bf16 exponent range absorbs the drift,
  and per-segment log-gains telescope through boundary column sums.
  Segments are packed 2-per-partition-group x 6-per-column-group into
  C=2 independent chains of (112, 384) matmul+multiply steps, with an
  exact side chain (48, 64) covering t in [0, 56) to anchor the
  telescoped magnitude.  The stationary matrix carries extra columns
  that compute end-capture rows and column sums for free; those rows
  ride through the F-multiply (F rows 48-63 are 1.0 via natfb padding)
  and are staged to DRAM, reloaded in (slot, seg*batch) layout, and
  selected by per-row length indicators.
  F tiles are produced by XBAR DMA transposes (128-source-column tiles)
  of a pre-exponentiated, 64-element-padded bf16 copy of feats (natfb),
  so the PE does no transposes and the ACT does no PSUM evacuation.
  Gold (numerator): one-hot tiles (bf16 tensor_scalar is_equal) over
  127-step chunks; bigram counts via offset-partition matmuls
  C += ohu[0:127]^T @ ohu[1:128] (mask baked into tags, so the pair
  weight mask_t*mask_{t+1} = mask_{t+1} is automatic); feature gathers
  via fused scalar_tensor_tensor ops split across DVE and Pool.
"""

import math

import numpy as np

import concourse.bacc as bacc
import concourse.mybir as mybir
import concourse.tile as tile
from concourse.bass import AP
from concourse.bass_utils import run_bass_kernel_spmd

F32 = mybir.dt.float32
BF16 = mybir.dt.bfloat16
I32 = mybir.dt.int32
AF = mybir.ActivationFunctionType
OP = mybir.AluOpType

B_FULL = 512
N_CORES = 8
BC = B_FULL // N_CORES  # 64
L_FULL = 1024
T = 48

MU = 0.51
ALPHA = math.log(T)
SEGLEN = 43
NSEG = 24          # segments s = 1..24, seg s main range [43s, 43s+43)
W = 12             # warmup steps
SLOTS = 56         # tau = 0..55; mm steps tau = 1..56
NTB = 576          # natfb window width in t-slots
W0 = 0             # chain-0 window start (t)
W1 = 508           # chain-1 window start (t)
NCH = 6            # column groups (m) per chain
WCH = NCH * BC     # 384 = chain column width
GOLD_CK = 8        # 128-step gold chunks


def build_program(dbg=False):
    L = L_FULL
    Bc = BC
    nc = bacc.Bacc("TRN2", target_bir_lowering=False, debug=False)

    feats_d = nc.dram_tensor("feats", (Bc, L, T), F32, kind="ExternalInput")
    trans_d = nc.dram_tensor("trans", (T, T), F32, kind="ExternalInput")
    start_d = nc.dram_tensor("start", (T,), F32, kind="ExternalInput")
    end_d = nc.dram_tensor("end", (T,), F32, kind="ExternalInput")
    tags_d = nc.dram_tensor("tags", (Bc, L), I32, kind="ExternalInput")
    mask_d = nc.dram_tensor("mask", (Bc, L), I32, kind="ExternalInput")
    out_d = nc.dram_tensor("out", (1, 1), F32, kind="ExternalOutput")
    dbg_d = (nc.dram_tensor("dbg", (8, Bc), F32, kind="ExternalOutput")
             if dbg else None)

    feats_flat = feats_d.ap().rearrange("b l t -> b (l t)")

    with tile.TileContext(nc) as tc:
        with (
            tc.tile_pool(name="const", bufs=1) as cp,
            tc.tile_pool(name="cps", bufs=1, space="PSUM") as cpp,
            tc.tile_pool(name="dramp", bufs=1, space="DRAM") as dp,
        ):
            # ---------------- constants ----------------
            iota48i = cp.tile((128, T), I32)
            nc.gpsimd.iota(iota48i[:, :], [[1, T]], channel_multiplier=0)
            iota48f = cp.tile((128, T), F32)
            nc.vector.tensor_copy(iota48f[:, :], iota48i[:, :])
            iota48b = cp.tile((128, T), BF16)
            nc.vector.tensor_copy(iota48b[:, :], iota48i[:, :])

            iota64i = cp.tile((64, 64), I32)
            nc.gpsimd.iota(iota64i[:, :], [[1, 64]], channel_multiplier=0)
            iotaPi = cp.tile((64, 1), I32)
            nc.gpsimd.iota(iotaPi[:, :], [[1, 1]], channel_multiplier=1)
            iota64f = cp.tile((64, 64), F32)
            nc.vector.tensor_copy(iota64f[:, :], iota64i[:, :])
            iotaPf = cp.tile((64, 1), F32)
            nc.vector.tensor_copy(iotaPf[:, :], iotaPi[:, :])
            identMf = cp.tile((64, 64), F32)
            nc.vector.tensor_scalar(
                identMf[:, :], iota64f[:, :], iotaPf[:, :], None, OP.is_equal)
            identMb = cp.tile((64, 64), BF16)
            nc.vector.tensor_copy(identMb[:, :], identMf[:, :])

            iotaLf = cp.tile((Bc, L), F32)

            ones128f = cp.tile((128, 1), F32)
            nc.vector.memset(ones128f[:, :], 1.0)
            ones128b = cp.tile((128, 1), BF16)
            nc.vector.memset(ones128b[:, :], 1.0)

            bias_mu = cp.tile((128, 1), F32)
            nc.vector.memset(bias_mu[:, :], -MU)
            bias_a = cp.tile((T, 1), F32)
            nc.vector.memset(bias_a[:, :], -ALPHA)

            # ---------------- params ----------------
            trans_sb = cp.tile((T, T), F32)
            nc.sync.dma_start(trans_sb[:, :], trans_d.ap())
            e_f32 = cp.tile((T, T), F32)
            nc.scalar.activation(e_f32[:, :], trans_sb[:, :], AF.Exp,
                                 bias=bias_a[:, :])
            e_b = cp.tile((T, T), BF16)
            nc.vector.tensor_copy(e_b[:, :], e_f32[:, :])

            end_sb = cp.tile((T, 1), F32)
            nc.sync.dma_start(end_sb[:, :], end_d.ap().unsqueeze(1))
            expend_f = cp.tile((T, 1), F32)
            nc.scalar.activation(expend_f[:, :], end_sb[:, :], AF.Exp)
            expend_b = cp.tile((T, 1), BF16)
            nc.vector.tensor_copy(expend_b[:, :], expend_f[:, :])

            start_sb = cp.tile((T, 1), F32)
            nc.sync.dma_start(start_sb[:, :], start_d.ap().unsqueeze(1))
            expstart = cp.tile((T, 1), F32)
            nc.scalar.activation(expstart[:, :], start_sb[:, :], AF.Exp)

            startbc = cp.tile((Bc, T), F32)
            nc.sync.dma_start(
                startbc[:, :], start_d.ap().unsqueeze(0).partition_broadcast(Bc))
            endbc = cp.tile((Bc, T), F32)
            nc.sync.dma_start(
                endbc[:, :], end_d.ap().unsqueeze(0).partition_broadcast(Bc))

            # main stationary (112, 112):
            #  rows 0-47 (block A), rows 64-111 (block B), rows 48-63 zero
            #  cols 0-47 = E(A), 64-111 = E(B), 48 = capA, 49 = capB,
            #  50 = sumA, 51 = sumB, 52-63 zero
            s_main = cp.tile((112, 112), BF16)
            nc.vector.memset(s_main[:, :], 0.0)
            nc.vector.tensor_copy(s_main[0:T, 0:T], e_b[:, :])
            nc.sync.dma_start(s_main[64:112, 64:112], e_b[:, :])
            nc.vector.tensor_copy(s_main[0:T, 48:49], expend_b[:, :])
            nc.sync.dma_start(s_main[64:112, 49:50], expend_b[:, :])
            nc.vector.memset(s_main[0:T, 50:51], 1.0)
            nc.vector.memset(s_main[64:112, 51:52], 1.0)

            # side stationary (48, 50): cols 0-47 E, 48 = cap, 49 = sum
            s_side = cp.tile((T, 50), BF16)
            nc.vector.tensor_copy(s_side[:, 0:T], e_b[:, :])
            nc.vector.tensor_copy(s_side[:, 48:49], expend_b[:, :])
            nc.vector.memset(s_side[:, 49:50], 1.0)

            # ---------------- tags / mask prep ----------------
            prep_scope = tc.tile_pool(name="prepsb", bufs=1)
            prp = prep_scope.__enter__()
            iotaLi = prp.tile((Bc, L), I32)
            nc.gpsimd.iota(iotaLi[:, :], [[1, L]], channel_multiplier=0)
            nc.vector.tensor_copy(iotaLf[:, :], iotaLi[:, :])
            tags_i = prp.tile((Bc, L), I32)
            nc.sync.dma_start(tags_i[:, :], tags_d.ap())
            tagsf = cp.tile((Bc, L), F32)
            nc.vector.tensor_copy(tagsf[:, :], tags_i[:, :])
            mask_i = prp.tile((Bc, L), I32)
            nc.sync.dma_start(mask_i[:, :], mask_d.ap())
            maskf = prp.tile((Bc, L), F32)
            nc.vector.tensor_copy(maskf[:, :], mask_i[:, :])
            tagsmb = prp.tile((Bc, L), BF16)
            moff = prp.tile((Bc, L), F32)
            nc.vector.tensor_scalar(moff[:, :], maskf[:, :], -100.0, 100.0,
                                    OP.mult, OP.add)
            tagsm_f = prp.tile((Bc, L), F32)
            nc.vector.tensor_tensor(tagsm_f[:, :], tagsf[:, :], moff[:, :],
                                    OP.add)
            nc.vector.tensor_copy(tagsmb[:, :], tagsm_f[:, :])

            tagsmSb = prp.tile((Bc, L), BF16)
            nc.vector.memset(tagsmSb[:, :], 100.0)
            nc.vector.tensor_copy(tagsmSb[:, 0:L - 1], tagsm_f[:, 1:L])

            lenb = cp.tile((Bc, 1), F32)
            nc.vector.tensor_reduce(lenb[:, :], maskf[:, :],
                                    mybir.AxisListType.X, OP.add)

            # transposed masked tags: 9 tiles (128, 64), 127-stride chunks
            tagsTm = []
            tagsTmS = []
            with tc.tile_pool(name="prepps", bufs=2, space="PSUM") as ppp:
                for k in range(GOLD_CK):
                    ps = ppp.tile((128, Bc), BF16, name=f"tps_{k}", tag="tps",
                                  bufs=2)
                    nc.tensor.transpose(ps[:, :],
                                        tagsmb[:, 128 * k:128 * (k + 1)],
                                        identMb[:, :])
                    tt = cp.tile((128, Bc), BF16, name=f"tagsTm_{k}")
                    nc.vector.tensor_copy(tt[:, :], ps[:, :])
                    tagsTm.append(tt)
                    ps2 = ppp.tile((128, Bc), BF16, name=f"tps2_{k}",
                                   tag="tps", bufs=2)
                    nc.tensor.transpose(ps2[:, :],
                                        tagsmSb[:, 128 * k:128 * (k + 1)],
                                        identMb[:, :])
                    tt2 = cp.tile((128, Bc), BF16, name=f"tagsTmS_{k}")
                    nc.vector.tensor_copy(tt2[:, :], ps2[:, :])
                    tagsTmS.append(tt2)
                # len row (1, 64) via transpose
                lps = ppp.tile((1, Bc), F32, name="lps", tag="lps", bufs=1)
                nc.tensor.transpose(lps[:, :], lenb[:, :], identMf[:, :])
                lenrow = cp.tile((1, Bc), F32)
                nc.vector.tensor_copy(lenrow[:, :], lps[:, :])
            prep_scope.__exit__(None, None, None)

            # (natfb/natfbS/A tiles are allocated inside the scan scope
            # below so their SBUF frees before the end phase)
            # ---------------- natfb: padded exp'd bf16 feats ----------------
            # (128, 56*12*64): row c*64+b holds chain c; column layout
            # (tau*12 + strip)*64 + jj with strip = m*2 + tp, so each
            # XBAR transpose slab input is CONTIGUOUS and 128-element
            # source groups give partitions tp*64 + jj.  jj 48-63 = 1.0
            # (become the F=1 ride-through rows after transpose).
            HA, HB = 32, 24  # natfb tau-halves: A = tau [0,32), B = [32,56)
            bigp_scope = tc.tile_pool(name="bigp", bufs=1)
            bigp = bigp_scope.__enter__()
            natfbA = bigp.tile((128, HA * 12 * 64), BF16)
            natfbB = bigp.tile((128, HB * 12 * 64), BF16)
            for nt, nh in ((natfbA, HA), (natfbB, HB)):
                nc.gpsimd.memset(
                    nt[:, :].rearrange("p (ts jj) -> p ts jj", ts=nh * 12,
                                       jj=64)[:, :, T:64], 1.0)
            # invalid tails of the two clipped strips (chain 1, s=23, 24)
            # strip (tp=1, m=4) -> strip idx 9: slots tau >= 47 invalid
            # strip (tp=1, m=5) -> strip idx 11: slots tau >= 4 invalid
            nc.gpsimd.memset(
                natfbB[64:128, :].rearrange(
                    "p (t s jj) -> p t s jj", t=HB, s=12,
                    jj=64)[:, 47 - HA:HB, 9, 0:T], 1.0)
            nc.gpsimd.memset(
                natfbA[64:128, :].rearrange(
                    "p (t s jj) -> p t s jj", t=HA, s=12,
                    jj=64)[:, 4:HA, 11, 0:T], 1.0)
            nc.gpsimd.memset(
                natfbB[64:128, :].rearrange(
                    "p (t s jj) -> p t s jj", t=HB, s=12,
                    jj=64)[:, :, 11, 0:T], 1.0)
            # side-chain feats: plain t-slot layout, t in [0, 56)
            natfbS = bigp.tile((Bc, SLOTS * 64), BF16)
            nc.gpsimd.memset(
                natfbS[:, :].rearrange("p (t jj) -> p t jj", t=SLOTS,
                                       jj=64)[:, :, T:64], 1.0)

            # stage DRAM: per chain (4, 57*384) bf16
            stage_dr = [dp.tile((4, 57 * WCH), BF16, name=f"stage_{c}")
                        for c in range(2)]

            with (
                tc.tile_pool(name="chkp", bufs=3) as chp,
                tc.tile_pool(name="fslab", bufs=2) as fsp,
                tc.tile_pool(name="fside", bufs=2) as fsdp,
                tc.tile_pool(name="scanps", bufs=3, space="PSUM") as sps,
                tc.tile_pool(name="scanps2", bufs=3, space="PSUM") as sps2,
                tc.tile_pool(name="sideps", bufs=1, space="PSUM") as sdps,
                tc.tile_pool(name="ohp", bufs=3) as ohp,
                tc.tile_pool(name="bounce", bufs=1) as bpp,
                tc.tile_pool(name="fgp", bufs=2) as fgp,
                tc.tile_pool(name="scrp", bufs=2) as scrp,
            ):
                # ---- feats strip DMAs + exp into natfb halves ----
                natfb4 = {}
                natfb4c1 = {}
                for nt, nh, hh in ((natfbA, HA, 0), (natfbB, HB, 1)):
                    natfb4[hh] = nt[:, :].rearrange(
                        "p (t s jj) -> p t s jj", t=nh, s=12, jj=64)
                    natfb4c1[hh] = nt[64:128, :].rearrange(
                        "p (t s jj) -> p t s jj", t=nh, s=12, jj=64)
                HOFF = {0: 0, 1: HA}
                HLEN = {0: HA, 1: HB}

                def emit_strip_half(tp, m, h):
                    # strip = m*2 + tp; seg s_c = 12c + 6tp + m + 1
                    strip = m * 2 + tp
                    s0 = 6 * tp + m + 1
                    s1 = s0 + 12
                    nvalid1 = min(SLOTS, max(0, L - (SEGLEN * s1 - 12)))
                    hl = HLEN[h]
                    t0 = SEGLEN * s0 - 12 + HOFF[h]
                    if nvalid1 == SLOTS:
                        ch = chp.tile((128, HA * T), F32, name="natf")
                        # split across 8 partition ranges -> 8 DMA engines
                        for pc in range(8):
                            b0 = pc * 8
                            nc.sync.dma_start(
                                ch[b0:b0 + 8, 0:hl * T],
                                AP(feats_flat.tensor,
                                   b0 * L * T + t0 * T,
                                   [[L * T, 8], [1, hl * T]]))
                            nc.sync.dma_start(
                                ch[64 + b0:64 + b0 + 8, 0:hl * T],
                                AP(feats_flat.tensor,
                                   b0 * L * T + (516 + t0) * T,
                                   [[L * T, 8], [1, hl * T]]))
                        nc.scalar.activation(
                            natfb4[h][:, :, strip, 0:T],
                            ch[:, 0:hl * T], AF.Exp, bias=bias_mu[:, :])
                    else:
                        ch = chp.tile((128, HA * T), F32, name="natf")
                        for pc in range(4):
                            b0 = pc * 16
                            in_ap = AP(feats_flat.tensor,
                                       b0 * L * T + t0 * T,
                                       [[L * T, 16], [1, hl * T]])
                            nc.sync.dma_start(
                                ch[b0:b0 + 16, 0:hl * T], in_ap)
                        nc.scalar.activation(
                            natfb4[h][0:Bc, :, strip, 0:T],
                            ch[0:Bc, 0:hl * T], AF.Exp,
                            bias=bias_mu[0:Bc, :])
                        nv = min(max(nvalid1 - HOFF[h], 0), hl)
                        if nv > 0:
                            ch2 = chp.tile((128, HA * T), F32,
                                           name="natf")
                            for pc in range(4):
                                b0 = pc * 16
                                in2 = AP(feats_flat.tensor,
                                         b0 * L * T +
                                         (SEGLEN * s1 - 12 + HOFF[h]) * T,
                                         [[L * T, 16], [1, nv * T]])
                                nc.sync.dma_start(
                                    ch2[b0:b0 + 16, 0:nv * T], in2)
                            nc.scalar.activation(
                                natfb4c1[h][:, 0:nv, strip, 0:T],
                                ch2[0:Bc, 0:nv * T], AF.Exp,
                                bias=bias_mu[0:Bc, :])

                # side strip first (unblocks the side chain)
                for h in range(2):
                    HSs = SLOTS // 2
                    chS = chp.tile((128, HA * T), F32, name="natf")
                    for pc in range(4):
                        b0 = pc * 16
                        nc.sync.dma_start(
                            chS[b0:b0 + 16, 0:HSs * T],
                            AP(feats_flat.tensor, b0 * L * T + h * HSs * T,
                               [[L * T, 16], [1, HSs * T]]))
                    nc.scalar.activation(
                        natfbS[:, :].rearrange(
                            "p (t jj) -> p t jj", t=SLOTS,
                            jj=64)[:, h * HSs:(h + 1) * HSs, 0:T],
                        chS[0:Bc, 0:HSs * T], AF.Exp,
                        bias=bias_mu[0:Bc, :])
                for h in range(2):
                    for m in range(NCH):
                        for tp in range(2):
                            emit_strip_half(tp, m, h)

                # ---- side chain (exact, t in [0, 56]) ----
                # F side slabs: q covers tau in [8q, 8q+8)
                side_slabs = {}

                natfbS_t = natfbS[:, :].tensor

                def emit_side_slab(q):
                    sl = fsdp.tile((128, 4 * 64), BF16, name="fside")
                    in_ap = AP(natfbS_t, 8 * q * 64,
                               [[SLOTS * 64, Bc], [1, 512]])
                    nc.scalar.dma_start_transpose(
                        sl[:, :].rearrange("p (e b) -> p e b", e=4, b=64),
                        in_ap)
                    side_slabs[q] = sl

                def side_f(tau):
                    sl = side_slabs[tau // 8]
                    p0 = (tau % 2) * 64
                    c0 = ((tau // 2) % 4) * 64
                    return sl[p0:p0 + 50, c0:c0 + 64]

                emit_side_slab(0)
                emit_side_slab(1)

                side_pool = tc.tile_pool(name="sidea", bufs=3)
                sap = side_pool.__enter__()
                a_side = sap.tile((50, Bc), BF16, name="a_side")
                # A_side(0) = exp(start) * F_0  (rows 48-49 will be junk)
                nc.vector.memset(a_side[32:50, :], 1.0)
                nc.vector.tensor_scalar(a_side[0:T, :],
                                        side_slabs[0][0:T, 0:64],
                                        expstart[:, :], None, OP.mult)
                lnsideS43 = cp.tile((1, Bc), F32)
                sums_side44 = cp.tile((1, Bc), BF16)

                for tau in range(1, 45):
                    if tau % 8 == 1 and tau // 8 + 2 <= 5:
                        emit_side_slab(tau // 8 + 2)
                    ps = sdps.tile((50, Bc), F32, name="side_ps")
                    nc.tensor.matmul(ps[:, :], s_side[:, :], a_side[0:T, :],
                                     start=True, stop=True,
                                     skip_group_check=True)
                    a_new = sap.tile((50, Bc), BF16, name="a_side")
                    nc.vector.tensor_tensor(a_new[:, :], ps[:, :],
                                            side_f(tau), OP.mult)
                    if tau == 44:
                        nc.sync.dma_start(sums_side44[:, :],
                                          a_new[49:50, :])
                    a_side = a_new
                nc.scalar.activation(lnsideS43[:, :], sums_side44[:, :],
                                     AF.Ln)
                side_pool.__exit__(None, None, None)

                # ---- main F slabs ----
                main_slabs = {}
                natfbA_t = natfbA[:, :].tensor
                natfbB_t = natfbB[:, :].tensor

                def emit_main_slab(c, q):
                    sl = fsp.tile((128, 8 * WCH), BF16, name="fslab")
                    nt, nh, qoff = ((natfbA_t, HA, 0) if q < 4
                                    else (natfbB_t, HB, 4))
                    if c == 0:
                        in_ap = AP(nt, (q - qoff) * 8 * 768,
                                   [[nh * 12 * 64, Bc], [1, 8 * 768]])
                    else:
                        # XBAR input must start at partition 0: bounce
                        # the chain-1 span down via an SBUF DMA first
                        bt = bpp.tile((Bc, 8 * 768), BF16, name="bounce")
                        nc.sync.dma_start(
                            bt[:, :],
                            AP(nt, 64 * (nh * 12 * 64) + (q - qoff)
                               * 8 * 768,
                               [[nh * 12 * 64, Bc], [1, 8 * 768]]))
                        in_ap = bt[:, :]
                    teng = nc.scalar if (c + q) % 2 == 0 else nc.sync
                    teng.dma_start_transpose(
                        sl[:, :].rearrange("p (e b) -> p e b", e=8 * NCH,
                                           b=64),
                        in_ap)
                    main_slabs[(c, q)] = sl

                for c in range(2):
                    emit_main_slab(c, 0)
                    emit_main_slab(c, 1)

                # ---- A ping-pong tiles ----
                app = [[bigp.tile((112, 8 * WCH), BF16, name=f"A_{c}_{i}")
                        for i in range(2)] for c in range(2)]
                for c in range(2):
                    t0 = app[c][0]
                    nc.vector.memset(t0[0:64, 0:WCH], 0.0)
                    nc.vector.memset(t0[0:52, 0:WCH], 1.0)
                    nc.vector.memset(t0[64:112, 0:WCH], 1.0)

                # ---- gold work generator (interleaved) ----
                c_ps = cpp.tile((T, T), F32, name="c_ps")
                feat_acc = cp.tile((128, 64), F32)
                nc.vector.memset(feat_acc[:, :], 0.0)
                # b-major iota: val[p, b*48+j] = j
                iota384i = cp.tile((128, 384), I32)
                nc.gpsimd.iota(iota384i[:, :], [[0, 8], [1, T]],
                               channel_multiplier=0)
                iota384b = cp.tile((128, 384), BF16)
                nc.vector.tensor_copy(iota384b[:, :], iota384i[:, :])

                gold_units = [(o, k) for o in range(8)
                              for k in range(GOLD_CK)]
                n_units = len(gold_units)
                gold_pos = [0]
                first_c = [True]

                def emit_gold(n):
                    for _ in range(n):
                        u = gold_pos[0]
                        if u >= n_units:
                            return
                        o, k = gold_units[u]
                        fg8 = fgp.tile((128, 384), F32, name="fg8")
                        in_ap = AP(feats_flat.tensor,
                                   8 * o * L * T + 128 * k * T,
                                   [[T, 128], [L * T, 8], [1, T]])
                        nc.gpsimd.dma_start(fg8[:, :], in_ap)
                        ohu8 = ohp.tile((128, 384), BF16, name="ohu8")
                        tu = tagsTm[k][:, :].tensor
                        nc.vector.tensor_tensor(
                            ohu8[:, :], iota384b[:, :],
                            AP(tu, 8 * o, [[Bc, 128], [1, 8], [0, T]]),
                            OP.is_equal)
                        ohs8 = ohp.tile((128, 384), BF16, name="ohs8")
                        ts_ = tagsTmS[k][:, :].tensor
                        nc.vector.tensor_tensor(
                            ohs8[:, :], iota384b[:, :],
                            AP(ts_, 8 * o, [[Bc, 128], [1, 8], [0, T]]),
                            OP.is_equal)
                        for b in range(8):
                            nc.tensor.matmul(
                                c_ps[:, :],
                                ohu8[:, b * T:(b + 1) * T],
                                ohs8[:, b * T:(b + 1) * T],
                                start=first_c[0], stop=False,
                                skip_group_check=True)
                            first_c[0] = False
                        scr = scrp.tile((128, 384), F32, name="scr",
                                        tag="scr")
                        nc.vector.scalar_tensor_tensor(
                            scr[:, :], ohu8[:, :], 1.0, fg8[:, :],
                            OP.mult, OP.mult,
                            accum_out=feat_acc[:, u:u + 1])
                        gold_pos[0] += 1

                # ---- main scan ----
                def a_slice(c, tau):
                    return app[c][(tau // 8) % 2][:, (tau % 8) * WCH:
                                                  (tau % 8 + 1) * WCH]

                def f_slice(c, tau):
                    # step 56 only needs the F=1 ride-through rows; reuse
                    # slot 55 (A rows 0-111 of state 56 are never used)
                    tau = min(tau, SLOTS - 1)
                    sl = main_slabs[(c, tau // 8)]
                    return sl[0:112, (tau % 8) * WCH:(tau % 8 + 1) * WCH]

                emit_gold(6)
                for tau in range(1, SLOTS + 1):
                    if tau % 8 == 1:
                        q = tau // 8 + 2
                        if q <= 6:
                            for c in range(2):
                                emit_main_slab(c, q)
                    for c in range(2):
                        pool = sps if c == 0 else sps2
                        ps = pool.tile((112, WCH), F32, name=f"mm_{c}")
                        nc.tensor.matmul(ps[:, :], s_main[:, :],
                                         a_slice(c, tau - 1)[0:112, :],
                                         start=True, stop=True,
                                         skip_group_check=True)
                        nc.vector.tensor_tensor(a_slice(c, tau), ps[:, :],
                                                f_slice(c, tau), OP.mult)
                    if tau % 8 == 7 or tau == SLOTS:
                        pass
                    if tau % 8 == 0:
                        q = tau // 8 - 1
                        for c in range(2):
                            nc.sync.dma_start(
                                stage_dr[c][0:4,
                                            q * 8 * WCH:(q + 1) * 8 * WCH],
                                app[c][q % 2][48:52, :])
                    emit_gold(2)
                # final partial block: slot 56 = A(56)
                for c in range(2):
                    nc.sync.dma_start(
                        stage_dr[c][0:4, 56 * WCH:57 * WCH],
                        app[c][1][48:52, 0:WCH])
                emit_gold(n_units)
                zrow = cp.tile((1, T), BF16)
                nc.vector.memset(zrow[:, :], 0.0)
                nc.tensor.matmul(c_ps[:, :], zrow[:, :], zrow[:, :],
                                 start=False, stop=True,
                                 skip_group_check=True)

                # ---- gold misc terms (start, end-transitions) ----
                misc_acc = cp.tile((Bc, 2), F32)
                scrb = scrp.tile((Bc, T), F32, name="scrb", tag="scrb")
                nc.vector.scalar_tensor_tensor(
                    scrb[:, :], iota48f[0:Bc, :], tagsf[:, 0:1],
                    startbc[:, :], OP.is_equal, OP.mult,
                    accum_out=misc_acc[:, 0:1])
                lm1 = cp.tile((Bc, 1), F32)
                nc.vector.tensor_scalar(lm1[:, :], lenb[:, :], 1.0, None,
                                        OP.subtract)
                scrLt = chp.tile((128, HA * T), F32, name="natf")
                scrL = scrLt[0:Bc, 0:L]
                lt = cp.tile((Bc, 1), F32)
                nc.vector.scalar_tensor_tensor(
                    scrL, iotaLf[:, :], lm1[:, :], tagsf[:, :],
                    OP.is_equal, OP.mult, accum_out=lt[:, :])
                scrb3 = scrp.tile((Bc, T), F32, name="scrb3", tag="scrb")
                nc.vector.scalar_tensor_tensor(
                    scrb3[:, :], iota48f[0:Bc, :], lt[:, :], endbc[:, :],
                    OP.is_equal, OP.mult, accum_out=misc_acc[:, 1:2])

            bigp_scope.__exit__(None, None, None)

            # =============== end phase ===============
            with (
                tc.tile_pool(name="endp", bufs=1) as ep,
                tc.tile_pool(name="endps", bufs=1, space="PSUM") as epp,
                tc.tile_pool(name="endps2", bufs=2, space="PSUM") as epp2,
            ):
                # gold assembly
                gold_ps = epp.tile((1, 1), F32, name="gold_ps")
                scrT = ep.tile((T, T), F32, name="scrT")
                cacc = ep.tile((T, 1), F32, name="cacc")
                nc.vector.tensor_tensor(scrT[:, :], c_ps[:, :],
                                        trans_sb[:, :], OP.mult)
                nc.vector.tensor_reduce(cacc[:, :], scrT[:, :],
                                        mybir.AxisListType.X, OP.add)
                nc.tensor.matmul(gold_ps[:, :], ones128f[0:T, :], cacc[:, :],
                                 start=True, stop=False,
                                 skip_group_check=True)
                fred = ep.tile((128, 1), F32, name="fred")
                nc.vector.tensor_reduce(fred[:, :], feat_acc[:, :],
                                        mybir.AxisListType.X, OP.add)
                nc.tensor.matmul(gold_ps[:, :], ones128f[:, :],
                                 fred[:, :], start=False, stop=False,
                                 skip_group_check=True)
                mred = ep.tile((Bc, 1), F32, name="mred")
                nc.vector.tensor_reduce(mred[:, :], misc_acc[:, :],
                                        mybir.AxisListType.X, OP.add)
                nc.tensor.matmul(gold_ps[:, :], ones128f[0:Bc, :],
                                 mred[:, :], start=False, stop=True,
                                 skip_group_check=True)

                # reload caps/sums: (57, 768) per chain
                capsre, sumsre, capsLn, sumsLn = [], [], [], []
                for c in range(2):
                    cr = ep.tile((57, 2 * WCH), BF16, name=f"capsre_{c}")
                    sr = ep.tile((57, 2 * WCH), BF16, name=f"sumsre_{c}")
                    st_t = stage_dr[c][:, :].tensor
                    nc.sync.dma_start(
                        cr[:, :], AP(st_t, 0,
                                     [[WCH, 57], [57 * WCH, 2],
                                      [64, NCH], [1, 64]]))
                    nc.sync.dma_start(
                        sr[:, :], AP(st_t, 2 * 57 * WCH,
                                     [[WCH, 57], [57 * WCH, 2],
                                      [64, NCH], [1, 64]]))
                    cl = ep.tile((57, 2 * WCH), F32, name=f"capsLn_{c}")
                    sl_ = ep.tile((57, 2 * WCH), F32, name=f"sumsLn_{c}")
                    nc.scalar.activation(cl[:, :], cr[:, :], AF.Ln)
                    nc.scalar.activation(sl_[:, :], sr[:, :], AF.Ln)
                    capsre.append(cr)
                    sumsre.append(sr)
                    capsLn.append(cl)
                    sumsLn.append(sl_)

                # lenrep (1, 1536) f32
                lenrep = ep.tile((1, 24 * 64), F32, name="lenrep")
                nc.vector.tensor_copy(lenrep[:, 0:64], lenrow[:, :])
                for w_ in (64, 128, 256, 512):
                    nc.vector.tensor_copy(lenrep[:, w_:2 * w_],
                                          lenrep[:, 0:w_])
                nc.vector.tensor_copy(lenrep[:, 1024:1536],
                                      lenrep[:, 0:512])

                # per-chain sigma indicator + capture select
                ones_row = ep.tile((1, 64), F32, name="ones_row")
                nc.vector.memset(ones_row[:, :], 1.0)
                comb = ep.tile((1, 24 * 64), F32, name="comb")
                for c in range(2):
                    io = ep.tile((57, 2 * WCH), I32, name=f"indio_{c}")
                    nc.gpsimd.iota(io[:, :], [[258, 2], [43, NCH], [0, 64]],
                                   channel_multiplier=1)
                    iof = ep.tile((57, 2 * WCH), F32,
                                  name=f"indiof_{c}")
                    nc.vector.tensor_copy(iof[:, :], io[:, :])
                    nc.vector.memset(iof[0:13, :], -9999.0)
                    if c == 0:
                        negrow = ep.tile((1, 2 * WCH), F32, name="negrow")
                        nc.vector.memset(negrow[:, :], -9999.0)
                    nc.sync.dma_start(iof[56:57, :], negrow[:, :])
                    lr_c = ep.tile((1, 2 * WCH), F32, name=f"lrc_{c}")
                    nc.vector.tensor_scalar(
                        lr_c[:, :], lenrep[:, 0:2 * WCH],
                        float(31 + 516 * c), None, OP.subtract)
                    for h in range(2):
                        hs = slice(h * WCH, (h + 1) * WCH)
                        lps_c = epp2.tile((57, WCH), F32,
                                          name=f"lps_{c}_{h}", tag="lps",
                                          bufs=2)
                        nc.tensor.matmul(lps_c[:, :], ones_row[0:1, 0:57],
                                         lr_c[:, hs], start=True, stop=True,
                                         skip_group_check=True)
                        ind = ep.tile((57, WCH), F32,
                                      name=f"ind_{c}_{h}")
                        nc.vector.tensor_tensor(ind[:, :], iof[:, hs],
                                                lps_c[:, :], OP.is_equal)
                        pr = ep.tile((57, WCH), F32,
                                      name=f"pr_{c}_{h}")
                        nc.vector.tensor_tensor(pr[:, :], capsLn[c][:, hs],
                                                ind[:, :], OP.mult)
                        fsel = epp2.tile((1, WCH), F32,
                                         name=f"fsel_{c}_{h}", tag="fsel",
                                         bufs=2)
                        nc.tensor.matmul(fsel[:, :], ones128f[0:57, :],
                                         pr[:, :], start=True, stop=True,
                                         skip_group_check=True)
                        nc.vector.tensor_copy(
                            comb[:, c * 768 + h * WCH:
                                 c * 768 + (h + 1) * WCH], fsel[:, :])
                Gall = ep.tile((1, 24 * 64), F32, name="Gall")
                s13 = ep.tile((1, 24 * 64), F32, name="s13")
                s56 = ep.tile((1, 24 * 64), F32, name="s56")
                for c in range(2):
                    nc.sync.dma_start(s13[:, c * 768:(c + 1) * 768],
                                      sumsLn[c][13:14, :])
                    nc.sync.dma_start(s56[:, c * 768:(c + 1) * 768],
                                      sumsLn[c][56:57, :])
                nc.vector.tensor_tensor(Gall[:, :], s56[:, :], s13[:, :],
                                        OP.subtract)
                # ge[s] = (len-1 >= 43(s+1)) at pos (s, b)
                ioS = ep.tile((1, 24 * 64), I32, name="ioS")
                nc.gpsimd.iota(ioS[:, :], [[43, 24], [0, 64]],
                               channel_multiplier=0)
                ioSf = ep.tile((1, 24 * 64), F32, name="ioSf")
                nc.vector.tensor_copy(ioSf[:, :], ioS[:, :])
                # ge1[g] = (s* >= g+1), ge2[g] = (s* >= g+2) with
                # s* = (len-1)//43; include G of seg s=g+1 iff ge2;
                # one-hot of s* group = ge1 - ge2
                lm44 = ep.tile((1, 24 * 64), F32, name="lm44")
                nc.vector.tensor_scalar(lm44[:, :], lenrep[:, :], 44.0,
                                        None, OP.subtract)
                ge1 = ep.tile((1, 24 * 64), F32, name="ge1")
                nc.vector.tensor_tensor(ge1[:, :], lm44[:, :], ioSf[:, :],
                                        OP.is_ge)
                lm87 = ep.tile((1, 24 * 64), F32, name="lm87")
                nc.vector.tensor_scalar(lm87[:, :], lenrep[:, :], 87.0,
                                        None, OP.subtract)
                ge2 = ep.tile((1, 24 * 64), F32, name="ge2")
                nc.vector.tensor_tensor(ge2[:, :], lm87[:, :], ioSf[:, :],
                                        OP.is_ge)
                ohsel = ep.tile((1, 24 * 64), F32, name="ohsel")
                nc.vector.tensor_tensor(ohsel[:, :], ge1[:, :], ge2[:, :],
                                        OP.subtract)
                tmp = ep.tile((1, 24 * 64), F32, name="tmp")
                nc.vector.tensor_tensor(tmp[:, :], ge2[:, :], Gall[:, :],
                                        OP.mult)
                nc.vector.tensor_tensor(comb[:, :], comb[:, :], tmp[:, :],
                                        OP.add)
                nc.vector.tensor_tensor(tmp[:, :], ohsel[:, :], s13[:, :],
                                        OP.mult)
                nc.vector.tensor_tensor(comb[:, :], comb[:, :], tmp[:, :],
                                        OP.subtract)
                # fold 24 groups -> 1
                for span in (768, 384, 192, 128, 64):
                    if span == 128:
                        nc.vector.tensor_tensor(comb[:, 0:64], comb[:, 0:64],
                                                comb[:, 128:192], OP.add)
                    else:
                        nc.vector.tensor_tensor(comb[:, 0:span],
                                                comb[:, 0:span],
                                                comb[:, span:2 * span],
                                                OP.add)
                # fwd = comb + lnsideS43 + len*(MU+ALPHA) - ALPHA
                fwd = ep.tile((1, Bc), F32, name="fwd")
                nc.vector.tensor_tensor(fwd[:, :], comb[:, 0:64],
                                        lnsideS43[:, :], OP.add)
                shifts = ep.tile((1, Bc), F32, name="shifts")
                nc.vector.tensor_scalar(shifts[:, :], lenrow[:, :],
                                        MU + ALPHA, -ALPHA, OP.mult, OP.add)
                nc.vector.tensor_tensor(fwd[:, :], fwd[:, :], shifts[:, :],
                                        OP.add)
                fwd_tot = ep.tile((1, 1), F32, name="fwd_tot")
                nc.vector.tensor_reduce(fwd_tot[:, :], fwd[:, :],
                                        mybir.AxisListType.X, OP.add)
                loss = ep.tile((1, 1), F32, name="loss")
                nc.vector.tensor_tensor(loss[:, :], fwd_tot[:, :],
                                        gold_ps[:, :], OP.subtract)
                nc.sync.dma_start(out_d.ap(), loss[:, :])
                if dbg:
                    gsb = ep.tile((1, 1), F32, name="gsb")
                    nc.scalar.copy(gsb[:, :], gold_ps[:, :])
                    nc.sync.dma_start(dbg_d.ap()[0:1, :], fwd[:, :])
                    nc.sync.dma_start(dbg_d.ap()[1:2, :], lnsideS43[:, :])
                    nc.sync.dma_start(dbg_d.ap()[2:3, :], lenrow[:, :])
                    nc.sync.dma_start(dbg_d.ap()[3:4, :], comb[:, 0:64])
                    nc.sync.dma_start(dbg_d.ap()[4:5, 0:1], gsb[:, :])
                    nc.sync.dma_start(dbg_d.ap()[5:6, :],
                                      comb[:, 0:64])
                    s44f = ep.tile((1, Bc), F32, name="s44f")
                    nc.vector.tensor_copy(s44f[:, :], sums_side44[:, :])
                    nc.sync.dma_start(dbg_d.ap()[6:7, :], s44f[:, :])
                    nc.sync.dma_start(dbg_d.ap()[7:8, :], lenrep[:, 0:64])

    nc.compile()
    return nc


def shard_inputs(feats, transitions, start_transitions, end_transitions,
                 tags, mask, n_cores=N_CORES):
    feats = np.ascontiguousarray(np.asarray(feats, dtype=np.float32))
    transitions = np.ascontiguousarray(
        np.asarray(transitions, dtype=np.float32))
    start_transitions = np.ascontiguousarray(
        np.asarray(start_transitions, dtype=np.float32))
    end_transitions = np.ascontiguousarray(
        np.asarray(end_transitions, dtype=np.float32))
    tags = np.ascontiguousarray(np.asarray(tags).astype(np.int32))
    mask = np.ascontiguousarray(np.asarray(mask).astype(np.int32))
    Bc = feats.shape[0] // n_cores
    in_maps = []
    for c in range(n_cores):
        s = slice(c * Bc, (c + 1) * Bc)
        in_maps.append({
            "feats": feats[s],
            "trans": transitions,
            "start": start_transitions,
            "end": end_transitions,
            "tags": tags[s],
            "mask": mask[s],
        })
    return in_maps, feats.shape


def kernel(feats, transitions, start_transitions, end_transitions, tags,
           mask, **_ignored):
    in_maps, _ = shard_inputs(
        feats, transitions, start_transitions, end_transitions, tags, mask)
    nc = build_program()
    res = run_bass_kernel_spmd(nc, in_maps, core_ids=list(range(N_CORES)))
    total = sum(float(r["out"][0, 0]) for r in res.results)
    return np.float32(total)
